# revision 24
# baseline (speedup 1.0000x reference)
"""BiLSTM-CRF (nn_BiLSTM_CRF_71141838291390) Trainium2 Bass kernel.

Two launches on the 8 axon'd NeuronCores:

Launch 1 (8 cores, SPMD, data-parallel over the 4096-token sentence):
  - core k owns t in [512k, 512k+512)
  - embedding gather on-device (indirect DMA from the zero-row-padded table)
  - xg = x @ w_ih.T via PE matmuls (weights transposed on device via PE)
  - the sequential LSTM recurrences are parallelized with chunked warmup:
    32 chains x (16 real + 32 warmup) steps per direction per core; the LSTM
    forgets its initial state geometrically (~0.4^32 ~ 1e-13 < f32 ulp), so
    warmed-up chains bit-lock onto the exact trajectory.  All 32 chains
    advance in lockstep inside each instruction (the per-step cost is
    dominated by reloading w_hh into the PE array and is amortized over all
    chains); fwd and bwd run as two interleaved groups so PE/ACT/DVE overlap.
    Sequence edges are handled exactly by forcing gate pre-activations
    (sigmoid(-1e4)=0) via an additive xg patch.
  - feats = concat(hf,hb) @ W_tag.T + b_tag, emitted per core as [512, 5]

Launch 2 (1 core): exact Viterbi.
  - forward max-plus scan over all 4096 steps, sequential, with the exact
    same op order/rounding as the reference (scores = fv + trans; max; +feat)
    -- required because the reference hits exact f32 ties whose index-order
    tie-break cannot be reproduced by any value-offset/chunked scheme.
  - backpointers are NOT stored; the backward pass is chunk-parallel
    (128 chunks x 32 steps on partitions), recomputing argmax rows from the
    stored (exact) fv history, producing candidate paths for all 5 possible
    chunk-end tags plus a link table.  The final 128-step integer walk is
    done on host.

kernel() is self-contained: hardcodes shapes, builds host-side constants from
the inputs, runs both launches, and stitches score+path.
"""

import os
import numpy as np

import concourse.bass as bass
import concourse.bacc as bacc
import concourse.mybir as mybir
from concourse import bass_utils
from concourse.tile import TileContext
from concourse.masks import make_identity

F32 = mybir.dt.float32
I32 = mybir.dt.int32
AX = mybir.AxisListType
OP = mybir.AluOpType
ACTF = mybir.ActivationFunctionType

# problem constants
V, E, H, T = 50000, 256, 512, 5
HD = H // 2
S = 4096
START, STOP = 3, 4
NEG = -10000.0

NCORE = 8
RLEN = S // NCORE              # 512 real positions per core
# LSTM chunking (launch 1)
NCH = 32                       # chains per direction per core
LC = 16                        # real steps per chain
WL = 32                        # warmup steps per chain
SL = LC + WL                   # 48 executed steps per chain
SLOTS = SL + 1                 # h history slots (slot 0 = initial zeros)
XGC = 576                      # xg columns: t in [R0-32, R0+544)
XROWS = 640                    # gathered rows (5 x 128, tail unused)
# Viterbi backward chunking (launch 2)
CV = 128                       # chunks
LV2 = S // CV                  # 32 steps per chunk
BIG = 1024.0


# ======================================================================
# Launch 1: LSTM + feats (8 cores)
# ======================================================================

def _build1():
    nc = bacc.Bacc("TRN2", target_bir_lowering=False, debug=False)

    dt = nc.dram_tensor
    embed = dt("embed", [V + 1, E], F32, kind="ExternalInput").ap()
    idx = dt("idx", [XROWS], I32, kind="ExternalInput").ap()
    w_ih_f = dt("w_ih_f", [4 * HD, E], F32, kind="ExternalInput").ap()
    w_hh_f = dt("w_hh_f", [4 * HD, HD], F32, kind="ExternalInput").ap()
    w_ih_b = dt("w_ih_b", [4 * HD, E], F32, kind="ExternalInput").ap()
    w_hh_b = dt("w_hh_b", [4 * HD, HD], F32, kind="ExternalInput").ap()
    bias_f = dt("bias_f", [4 * HD], F32, kind="ExternalInput").ap()
    bias_b = dt("bias_b", [4 * HD], F32, kind="ExternalInput").ap()
    wtag = dt("wtag", [T, H], F32, kind="ExternalInput").ap()
    btag_rep = dt("btag_rep", [128, T], F32, kind="ExternalInput").ap()
    patch_f = dt("patch_f", [128, 8 * 32], F32, kind="ExternalInput").ap()
    patch_b = dt("patch_b", [128, 8 * 32], F32, kind="ExternalInput").ap()

    feats_out = dt("feats_out", [NCH, LC * T], F32, kind="ExternalOutput").ap()

    with TileContext(nc) as tc:
        with (
            tc.tile_pool(name="persist", bufs=1) as pp,
            tc.tile_pool(name="work", bufs=3) as wp,
        ):
            qA_cm = tc.tile_pool(name="psum", bufs=1, space="PSUM")
            qA = qA_cm.__enter__()
            ident = pp.tile([128, 128], F32, tag="ident")
            make_identity(nc, ident[:])
            idx_sb = pp.tile([128, 5], I32, tag="idx_sb")
            nc.sync.dma_start(out=idx_sb[:], in_=idx.rearrange("(t p) -> p t", p=128))
            btag_sb = pp.tile([128, T], F32, tag="btag_sb")
            nc.sync.dma_start(out=btag_sb[:], in_=btag_rep)
            bf_sb = pp.tile([128, 8], F32, tag="bf_sb")
            nc.sync.dma_start(out=bf_sb[:], in_=bias_f.rearrange("(j p) -> p j", p=128))
            bb_sb = pp.tile([128, 8], F32, tag="bb_sb")
            nc.sync.dma_start(out=bb_sb[:], in_=bias_b.rearrange("(j p) -> p j", p=128))
            pf_sb = pp.tile([128, 8 * 32], F32, tag="pf_sb")
            nc.sync.dma_start(out=pf_sb[:], in_=patch_f)
            pb_sb = pp.tile([128, 8 * 32], F32, tag="pb_sb")
            nc.sync.dma_start(out=pb_sb[:], in_=patch_b)

            # ---------------- phase A: all loads (DMA + gathers) -----------------
            x_rows = pp.tile([128, 5 * E], F32, tag="x_rows")
            xr3 = x_rows[:].rearrange("p (t e) -> p t e", t=5)
            for t in range(5):
                nc.gpsimd.indirect_dma_start(
                    out=xr3[:, t, :],
                    out_offset=None,
                    in_=embed,
                    in_offset=bass.IndirectOffsetOnAxis(ap=idx_sb[:, t : t + 1], axis=0),
                )
            wraws = {}
            for (wdram, nm) in ((w_ih_f, "wihraw_f"), (w_hh_f, "whhraw_f"),
                                (w_ih_b, "wihraw_b"), (w_hh_b, "whhraw_b")):
                wraw = pp.tile([128, 8 * 256], F32, tag=nm)
                nc.sync.dma_start(
                    out=wraw[:].rearrange("p (j e) -> p j e", j=8),
                    in_=wdram.rearrange("(j p) e -> p j e", p=128),
                )
                wraws[nm] = wraw
            wtag_raw = pp.tile([T, H], F32, tag="wtag_raw")
            nc.sync.dma_start(out=wtag_raw[:], in_=wtag)

            # ---------------- phase B: all PE transposes -----------------
            xT = pp.tile([128, 2 * XROWS], F32, tag="xT")
            xT3 = xT[:].rearrange("p (k r) -> p k r", k=2)
            for t in range(5):
                for ke in range(2):
                    ps = qA.tile([128, 288], F32, tag="psA", bufs=2, name="pst")[:, 0:128]
                    nc.tensor.transpose(
                        out=ps[:], in_=xr3[:, t, ke * 128 : (ke + 1) * 128], identity=ident[:]
                    )
                    nc.scalar.copy(out=xT3[:, ke, t * 128 : (t + 1) * 128], in_=ps[:])

            def transpose_w(wraw, tag):
                wr3 = wraw[:].rearrange("p (j e) -> p j e", j=8)
                wT = pp.tile([128, 2 * 8 * 128], F32, tag=tag)
                wT4 = wT[:].rearrange("p (k j e) -> p k j e", k=2, j=8)
                for j in range(8):
                    for ke in range(2):
                        ps = qA.tile([128, 288], F32, tag="psA", bufs=2, name="pst")[:, 0:128]
                        nc.tensor.transpose(
                            out=ps[:], in_=wr3[:, j, ke * 128 : (ke + 1) * 128], identity=ident[:]
                        )
                        nc.scalar.copy(out=wT4[:, ke, j, :], in_=ps[:])
                return wT

            wihT_f = transpose_w(wraws["wihraw_f"], "wihT_f")
            whhT_f = transpose_w(wraws["whhraw_f"], "whhT_f")
            wihT_b = transpose_w(wraws["wihraw_b"], "wihT_b")
            whhT_b = transpose_w(wraws["whhraw_b"], "whhT_b")

            wtagT = pp.tile([128, 4 * T], F32, tag="wtagT")
            wtagT3 = wtagT[:].rearrange("p (k n) -> p k n", k=4)
            for kt in range(4):
                ps = qA.tile([128, 288], F32, tag="psA", bufs=2, name="pst")[:, 0:128]
                nc.tensor.transpose(
                    out=ps[:, :T], in_=wtag_raw[:, kt * 128 : (kt + 1) * 128],
                    identity=ident[:T, :T],
                )
                nc.scalar.copy(out=wtagT3[:, kt, :], in_=ps[:, :T])

            # ---------------- xg = x @ w_ih.T + b  -----------------
            xg_f = pp.tile([128, 8 * XGC], F32, tag="xg_f")
            xg_b = pp.tile([128, 8 * XGC], F32, tag="xg_b")
            for (xg, wihT, b_sb) in ((xg_f, wihT_f, bf_sb), (xg_b, wihT_b, bb_sb)):
                xg3 = xg[:].rearrange("p (j t) -> p j t", j=8)
                wT4 = wihT[:].rearrange("p (k j e) -> p k j e", k=2, j=8)
                for rc in range(2):
                    for j in range(8):
                        ps = qA.tile([128, 288], F32, tag="psA", bufs=2, name="psxg")
                        for ke in range(2):
                            nc.tensor.matmul(
                                out=ps[:],
                                lhsT=wT4[:, ke, j, :],
                                rhs=xT3[:, ke, rc * 288 : (rc + 1) * 288],
                                start=(ke == 0),
                                stop=(ke == 1),
                            )
                        nc.vector.tensor_scalar_add(
                            out=xg3[:, j, rc * 288 : (rc + 1) * 288],
                            in0=ps[:],
                            scalar1=b_sb[:, j : j + 1],
                        )
            # pad patches (force h=c=0 outside the sequence; zeros on middle cores)
            xgf3 = xg_f[:].rearrange("p (j t) -> p j t", j=8)
            xgb3 = xg_b[:].rearrange("p (j t) -> p j t", j=8)
            nc.vector.tensor_add(
                out=xgf3[:, :, 0:32],
                in0=xgf3[:, :, 0:32],
                in1=pf_sb[:].rearrange("p (j t) -> p j t", j=8),
            )
            nc.vector.tensor_add(
                out=xgb3[:, :, XGC - 32 : XGC],
                in0=xgb3[:, :, XGC - 32 : XGC],
                in1=pb_sb[:].rearrange("p (j t) -> p j t", j=8),
            )
            # ---------------- LSTM recurrences (chunked, lockstep) -----------------
            hh_f = pp.tile([128, 2 * SLOTS * NCH], F32, tag="hh_f")
            hh_b = pp.tile([128, 2 * SLOTS * NCH], F32, tag="hh_b")
            hhf4 = hh_f[:].rearrange("p (k s c) -> p k s c", k=2, s=SLOTS)
            hhb4 = hh_b[:].rearrange("p (k s c) -> p k s c", k=2, s=SLOTS)
            c_f = pp.tile([128, 2 * NCH], F32, tag="c_f")
            c_b = pp.tile([128, 2 * NCH], F32, tag="c_b")
            nc.vector.memset(hhf4[:, :, 0, :], 0.0)
            nc.vector.memset(hhb4[:, :, 0, :], 0.0)
            nc.vector.memset(c_f[:], 0.0)
            nc.vector.memset(c_b[:], 0.0)

            # xg viewed as [p, j, colchunk(16-wide), u]: chain c at step s reads
            # column 16c + off  ->  slice [cq:cq+NCH] at u=off%16
            xgf4 = xg_f[:].rearrange("p (j c u) -> p j c u", j=8, u=16)
            xgb4 = xg_b[:].rearrange("p (j c u) -> p j c u", j=8, u=16)
            whf4 = whhT_f[:].rearrange("p (k j e) -> p k j e", k=2, j=8)
            whb4 = whhT_b[:].rearrange("p (k j e) -> p k j e", k=2, j=8)

            groups = (
                ("f", hhf4, c_f, whf4, xgf4),
                ("b", hhb4, c_b, whb4, xgb4),
            )
            for s in range(SL):
                for (gname, hh4, c_st, wh4, xg4) in groups:
                    col = s if gname == "f" else (79 - s)
                    cq, cu = col // 16, col % 16
                    ps = qA.tile([128, 8 * NCH], F32, tag=f"psg_{gname}", bufs=2)
                    for j in range(8):
                        for ke in range(2):
                            nc.tensor.matmul(
                                out=ps[:, j * NCH : (j + 1) * NCH],
                                lhsT=wh4[:, ke, j, :],
                                rhs=hh4[:, ke, s, :],
                                start=(ke == 0),
                                stop=(ke == 1),
                            )
                    pre = wp.tile([128, 8 * NCH], F32, tag=f"pre_{gname}")
                    nc.vector.tensor_add(
                        out=pre[:].rearrange("p (j c) -> p j c", j=8),
                        in0=ps[:].rearrange("p (j c) -> p j c", j=8),
                        in1=xg4[:, :, cq : cq + NCH, cu],
                    )
                    sif = wp.tile([128, 4 * NCH], F32, tag=f"sif_{gname}")
                    nc.scalar.activation(
                        out=sif[:], in_=pre[:, 0 : 4 * NCH], func=ACTF.Sigmoid,
                        bias=0.0, scale=1.0,
                    )
                    tg = wp.tile([128, 2 * NCH], F32, tag=f"tg_{gname}")
                    nc.scalar.activation(
                        out=tg[:], in_=pre[:, 4 * NCH : 6 * NCH], func=ACTF.Tanh,
                        bias=0.0, scale=1.0,
                    )
                    so = wp.tile([128, 2 * NCH], F32, tag=f"so_{gname}")
                    nc.scalar.activation(
                        out=so[:], in_=pre[:, 6 * NCH : 8 * NCH], func=ACTF.Sigmoid,
                        bias=0.0, scale=1.0,
                    )
                    ig = wp.tile([128, 2 * NCH], F32, tag=f"ig_{gname}")
                    nc.vector.tensor_mul(out=ig[:], in0=sif[:, 0 : 2 * NCH], in1=tg[:])
                    fc = wp.tile([128, 2 * NCH], F32, tag=f"fc_{gname}")
                    nc.vector.tensor_mul(out=fc[:], in0=sif[:, 2 * NCH : 4 * NCH], in1=c_st[:])
                    nc.vector.tensor_add(out=c_st[:], in0=fc[:], in1=ig[:])
                    tc_ = wp.tile([128, 2 * NCH], F32, tag=f"tc_{gname}")
                    nc.scalar.activation(
                        out=tc_[:], in_=c_st[:], func=ACTF.Tanh, bias=0.0, scale=1.0
                    )
                    nc.vector.tensor_mul(
                        out=hh4[:, :, s + 1, :],
                        in0=so[:].rearrange("p (k c) -> p k c", k=2),
                        in1=tc_[:].rearrange("p (k c) -> p k c", k=2),
                    )
            # ---------------- feats [chain, (step, tag)] -----------------
            psf = qA.tile([NCH, LC * T], F32, tag="psfeat", bufs=1)
            for r in range(LC):
                for kk in range(4):
                    if kk < 2:
                        lhsT = hhf4[:, kk, r + WL + 1, 0:NCH]
                    else:
                        lhsT = hhb4[:, kk - 2, SL - r, 0:NCH]
                    nc.tensor.matmul(
                        out=psf[:, r * T : (r + 1) * T],
                        lhsT=lhsT,
                        rhs=wtagT3[:, kk, :],
                        start=(kk == 0),
                        stop=(kk == 3),
                    )
            feats_sb = pp.tile([NCH, LC * T], F32, tag="feats_sb")
            nc.vector.tensor_add(
                out=feats_sb[:].rearrange("p (s n) -> p s n", n=T),
                in0=psf[:].rearrange("p (s n) -> p s n", n=T),
                in1=btag_sb[:NCH, :].rearrange("p (a n) -> p a n", a=1).to_broadcast([NCH, LC, T]),
            )
            nc.gpsimd.dma_start(out=feats_out, in_=feats_sb[:])
            qA_cm.__exit__(None, None, None)

    nc.finalize()
    return nc


# ======================================================================
# Launch 2: exact Viterbi (1 core)
# ======================================================================

def _build2():
    nc = bacc.Bacc("TRN2", target_bir_lowering=False, debug=False)

    dt = nc.dram_tensor
    feats_in = dt("feats_in", [S * T], F32, kind="ExternalInput").ap()
    trans_rep = dt("trans_rep", [128, T * T], F32, kind="ExternalInput").ap()
    iota25 = dt("iota25", [128, T * T], F32, kind="ExternalInput").ap()

    cand_out = dt("cand_out", [CV, LV2 * T], F32, kind="ExternalOutput").ap()
    link_out = dt("link_out", [CV, T], F32, kind="ExternalOutput").ap()
    term_out = dt("term_out", [1, T], F32, kind="ExternalOutput").ap()
    hist_scratch = dt("hist_scratch", [(S + 1) * T], F32, kind="ExternalOutput").ap()

    with TileContext(nc) as tc:
        with (
            tc.tile_pool(name="persist", bufs=1) as pp,
        ):
            trans_sb = pp.tile([128, 25], F32, tag="trans_sb")
            nc.sync.dma_start(out=trans_sb[:], in_=trans_rep)
            iota_sb = pp.tile([128, 25], F32, tag="iota_sb")
            nc.sync.dma_start(out=iota_sb[:], in_=iota25)
            feats_sb = pp.tile([1, S * T], F32, tag="feats_sb")
            nc.sync.dma_start(out=feats_sb[:], in_=feats_in.rearrange("(a x) -> a x", a=1))

            # ---------------- serial exact forward scan -----------------
            fvh = pp.tile([1, (S + 1) * T], F32, tag="fvh")
            fvh3 = fvh[:].rearrange("p (s n) -> p s n", n=T)
            nc.vector.memset(fvh[:, 0:T], NEG)
            nc.vector.memset(fvh[:, START : START + 1], 0.0)
            tr3 = trans_sb[:1, :].rearrange("p (n q) -> p n q", n=T)
            fe3 = feats_sb[:].rearrange("p (s n) -> p s n", n=T)
            scr = pp.tile([1, 25], F32, tag="scr")
            scr3 = scr[:].rearrange("p (n q) -> p n q", n=T)
            mx = pp.tile([1, T], F32, tag="mx")
            for t in range(S):
                nc.vector.tensor_add(
                    out=scr3[:],
                    in0=fvh3[:, t : t + 1, :].to_broadcast([1, T, T]),
                    in1=tr3[:],
                )
                nc.vector.tensor_reduce(out=mx[:], in_=scr3[:], axis=AX.X, op=OP.max)
                nc.vector.tensor_add(out=fvh3[:, t + 1, :], in0=mx[:], in1=fe3[:, t, :])
            nc.gpsimd.dma_start(out=term_out, in_=fvh3[:, S, :])

            # bounce history through DRAM to spread chunks across partitions
            nc.sync.dma_start(
                out=hist_scratch.rearrange("(a x) -> a x", a=1), in_=fvh[:]
            )
            hmain = pp.tile([CV, LV2 * T], F32, tag="hmain")
            nc.sync.dma_start(
                out=hmain[:],
                in_=hist_scratch[T : (S + 1) * T].rearrange("(c r) -> c r", c=CV),
            )
            hbnd = pp.tile([CV, T], F32, tag="hbnd")
            nc.sync.dma_start(
                out=hbnd[:],
                in_=hist_scratch[0 : S * T].rearrange("(c r) -> c r", c=CV)[:, 0:T],
            )
            hm3 = hmain[:].rearrange("p (s n) -> p s n", n=T)

            # ---------------- chunk-parallel backward (exact history) ----------
            cand = pp.tile([CV, LV2 * T], F32, tag="cand")
            cand3 = cand[:].rearrange("p (s n) -> p s n", n=T)
            link = pp.tile([CV, T], F32, tag="link")
            nc.vector.tensor_copy(out=cand3[:, LV2 - 1, :], in_=iota_sb[:CV, 0:T])
            scr2 = pp.tile([CV, 25], F32, tag="scr2")
            scr23 = scr2[:].rearrange("p (n q) -> p n q", n=T)
            mx2 = pp.tile([CV, T], F32, tag="mx2")
            eq = pp.tile([CV, 25], F32, tag="eq")
            eq3 = eq[:].rearrange("p (n q) -> p n q", n=T)
            tmp = pp.tile([CV, 25], F32, tag="tmpv")
            tmp3 = tmp[:].rearrange("p (n q) -> p n q", n=T)
            bpn = pp.tile([CV, T], F32, tag="bpn")
            oh = pp.tile([CV, 25], F32, tag="oh")
            oh3 = oh[:].rearrange("p (j n) -> p j n", j=T)
            prod = pp.tile([CV, 25], F32, tag="prod")
            prod3 = prod[:].rearrange("p (j n) -> p j n", j=T)
            csum = pp.tile([CV, T], F32, tag="csum")
            tr3b = trans_sb[:CV, :].rearrange("p (n q) -> p n q", n=T)
            iota_n = iota_sb[:CV, 0:T].rearrange("p (a n) -> p a n", a=1).to_broadcast([CV, T, T])
            for it in range(LV2):
                s = (LV2 - 1) - it          # 31 down to 0; global step t = 32c + s
                # fv before step t: s>=1 -> hmain[:, s-1, :]; s==0 -> hbnd
                fv_in = hm3[:, s - 1, :] if s >= 1 else hbnd[:]
                nc.vector.tensor_add(
                    out=scr23[:],
                    in0=fv_in.rearrange("p (a q) -> p a q", a=1).to_broadcast([CV, T, T]),
                    in1=tr3b[:],
                )
                nc.vector.tensor_reduce(out=mx2[:], in_=scr23[:], axis=AX.X, op=OP.max)
                nc.vector.tensor_tensor(
                    out=eq3[:],
                    in0=scr23[:],
                    in1=mx2[:].rearrange("p (n a) -> p n a", a=1).to_broadcast([CV, T, T]),
                    op=OP.is_equal,
                )
                nc.vector.scalar_tensor_tensor(
                    out=tmp[:], in0=eq[:], scalar=-BIG, in1=iota_sb[:CV, :],
                    op0=OP.mult, op1=OP.add,
                )
                nc.vector.tensor_reduce(out=bpn[:], in_=tmp3[:], axis=AX.X, op=OP.min)
                src = cand3[:, s, :]
                nc.vector.tensor_tensor(
                    out=oh3[:],
                    in0=src.rearrange("p (q a) -> p q a", a=1).to_broadcast([CV, T, T]),
                    in1=iota_n,
                    op=OP.is_equal,
                )
                nc.vector.tensor_mul(
                    out=prod3[:],
                    in0=oh3[:],
                    in1=bpn[:].rearrange("p (a n) -> p a n", a=1).to_broadcast([CV, T, T]),
                )
                nc.vector.tensor_reduce(out=csum[:], in_=prod3[:], axis=AX.X, op=OP.add)
                dst = link[:] if s == 0 else cand3[:, s - 1, :]
                nc.vector.tensor_scalar_add(out=dst, in0=csum[:], scalar1=BIG)

            nc.gpsimd.dma_start(out=cand_out, in_=cand[:])
            nc.gpsimd.dma_start(out=link_out, in_=link[:])

    nc.finalize()
    return nc


_NC1 = None
_NC2 = None


def _get_ncs():
    global _NC1, _NC2
    if _NC1 is None:
        _NC1 = _build1()
    if _NC2 is None:
        _NC2 = _build2()
    return _NC1, _NC2


LAST_RESULTS1 = None
LAST_RESULTS2 = None


def kernel(sentence, embed, w_ih_f, w_hh_f, b_f, w_ih_b, w_hh_b, b_b,
           W_tag, b_tag, transitions):
    global LAST_RESULTS1, LAST_RESULTS2
    f32 = np.float32
    sentence = np.asarray(sentence)
    embed = np.asarray(embed, f32)
    w_ih_f = np.ascontiguousarray(np.asarray(w_ih_f, f32))
    w_hh_f = np.ascontiguousarray(np.asarray(w_hh_f, f32))
    w_ih_b = np.ascontiguousarray(np.asarray(w_ih_b, f32))
    w_hh_b = np.ascontiguousarray(np.asarray(w_hh_b, f32))
    b_f = np.asarray(b_f, f32)
    b_b = np.asarray(b_b, f32)
    W_tag = np.ascontiguousarray(np.asarray(W_tag, f32))
    b_tag = np.asarray(b_tag, f32)
    transitions = np.asarray(transitions, f32)

    embed_pad = np.ascontiguousarray(
        np.concatenate([embed, np.zeros((1, E), f32)], axis=0)
    )

    # per-gate-tile pad constant: i,f,o -> -1e4 (sigmoid -> 0 exactly), g -> 0
    padconst = np.array([-1e4, -1e4, -1e4, -1e4, 0.0, 0.0, -1e4, -1e4], f32)
    btag_rep = np.tile(b_tag.reshape(1, T), (128, 1)).astype(f32)

    in_maps1 = []
    for k in range(NCORE):
        R0 = RLEN * k
        XG0 = R0 - 32
        tt = XG0 + np.arange(XROWS)
        idx = np.where((tt >= 0) & (tt < S), sentence[np.clip(tt, 0, S - 1)], V)
        idx = idx.astype(np.int32)

        pf = np.zeros((128, 8, 32), f32)
        pb = np.zeros((128, 8, 32), f32)
        if k == 0:
            pf = np.broadcast_to(
                padconst[None, :, None] - b_f.reshape(8, 128).T[:, :, None],
                (128, 8, 32)).astype(f32)
        if k == NCORE - 1:
            pb = np.broadcast_to(
                padconst[None, :, None] - b_b.reshape(8, 128).T[:, :, None],
                (128, 8, 32)).astype(f32)

        in_maps1.append({
            "embed": embed_pad,
            "idx": idx,
            "w_ih_f": w_ih_f, "w_hh_f": w_hh_f,
            "w_ih_b": w_ih_b, "w_hh_b": w_hh_b,
            "bias_f": b_f, "bias_b": b_b,
            "wtag": W_tag,
            "btag_rep": btag_rep,
            "patch_f": np.ascontiguousarray(pf.reshape(128, 8 * 32).astype(f32)),
            "patch_b": np.ascontiguousarray(pb.reshape(128, 8 * 32).astype(f32)),
        })

    nc1, nc2 = _get_ncs()
    trace = bool(int(os.environ.get("BASS_KERNEL_TRACE", "0")))
    res1 = bass_utils.run_bass_kernel_spmd(
        nc1, in_maps1, core_ids=list(range(NCORE)), trace=trace,
    )
    LAST_RESULTS1 = res1

    # feats_out per core: [32 chains, 16 steps, 5] -> [512, 5] in t order
    feats_full = np.concatenate(
        [res1.results[k]["feats_out"].reshape(RLEN, T) for k in range(NCORE)], axis=0
    ).astype(f32)

    trans_rep = np.tile(transitions.reshape(1, T * T), (128, 1)).astype(f32)
    iota25 = np.tile((np.arange(T * T, dtype=np.int64) % T).astype(f32), (128, 1))
    in_maps2 = [{
        "feats_in": np.ascontiguousarray(feats_full.reshape(S * T)),
        "trans_rep": trans_rep,
        "iota25": iota25.astype(f32),
    }]
    res2 = bass_utils.run_bass_kernel_spmd(nc2, in_maps2, core_ids=[0], trace=trace)
    LAST_RESULTS2 = res2

    cand = res2.results[0]["cand_out"].reshape(CV, LV2, T)
    link = res2.results[0]["link_out"]
    term = res2.results[0]["term_out"].reshape(T)

    terminal = term.astype(f32) + transitions[STOP]
    v = int(np.argmax(terminal))
    score = terminal[v]

    path = np.zeros(S, np.int64)
    for c in range(CV - 1, -1, -1):
        path[c * LV2 : (c + 1) * LV2] = np.rint(cand[c, :, v]).astype(np.int64)
        v = int(np.rint(link[c, v]))
    return np.float32(score), path.astype(np.int32)
